# revision 3
# baseline (speedup 1.0000x reference)
"""CompressiveMemory TRN2 kernel (8 NeuronCores, token-axis sharding).

reference computation (S=65536, D_IN=128, EK=EV=256):
    xc   = [relu(x), relu(-x)]                  # [S, 256]
    f    = xc * roll(xc, 1, axis=-1)            # dpfp, nu=1
    xa   = f / f.sum(-1)                        # sum-normalized
    retrieve(x) = (xa @ mem) / (xa @ zn.T + 1e-8)
    a_mem = retrieve(q); r_k = retrieve(k)
    new_memory = memory + k_act.T @ (v - r_k), k_act = act(k)
    new_z_norm = z_norm + k_act                 # [S, 256]

Sharding: S split across 8 cores; memory/z_norm replicated. Each core
produces its a_mem / new_z_norm shard plus a partial
  toadd_part = sum_tokens k_act.T @ (r_k - v)       (note: negated)
which the host combines: new_memory = memory - sum(parts).

Key identities used on-device (per 128-token chunk, tokens on partitions):
  num|den|rowsum = f @ [mem | zn.T | 1]   (one matmul, N=258; denominator
      and row-sum come out as free extra columns)
  a_mem = num * (1/den)            (the 1e-8 guard is dropped: den/rowsum
      is a nonneg-weighted average of zn~U[0,1], >= ~1e-3, so 1e-8
      perturbs by < 1e-5 relative -- below fp32 matmul noise)
  w'    = num * (1/den) - v        (fused scalar_tensor_tensor)
  k_act = f * (1/rowsum)
  nz    = k_act + zn               (broadcast add)
"""

import sys

sys.path.insert(0, "/opt/trn_rl_repo")

import numpy as np

import concourse.bass as bass
import concourse.mybir as mybir
import concourse.tile as tile
from concourse.bass_utils import run_bass_kernel_spmd
from concourse.masks import make_identity
from concourse.vector_clock import ScopedClock

F32 = mybir.dt.float32
AOP = mybir.AluOpType
AFT = mybir.ActivationFunctionType

N_CORES = 8
S, D_IN, EK, EV = 65536, 128, 256, 256
SC = S // N_CORES      # tokens per core
P = 128                # tokens per chunk (partition dim)
NCHUNK = SC // P       # 64
B = 8                  # chunks per superblock (elementwise batching)
NSB = NCHUNK // B      # 8


def _patched_drain_and_barrier(self, tick_clock, wait_clock):
    # This container's walrus build rejects Drain/NoOp instructions carrying
    # more than one sync wait ("Too many sync wait commands"). Split the
    # TileContext tail-drain waits across one NOP per proc.
    vc = tick_clock.global_clock
    for proc in range(len(vc)):
        tick = vc[proc]
        if tick > 0:
            nop = self.nc.sync.nop(nofuse=True, hint="tail_wait")
            req = ScopedClock()
            req.require_at_least(None, proc, tick)
            wait_clock.add_sem_waits(nop.ins, req)
    self.nc.sync.drain()
    self.nc.all_engine_barrier()
    assert self.sems is not None
    popped = self.nc._tile_sem_poison_stack.pop()
    assert popped is self._sem_poison
    self.nc.clear_and_free_semaphores(list(self.sems.allocated().values()))
    self.nc.all_engine_barrier()


tile.TileContext._drain_and_barrier = _patched_drain_and_barrier


def _split_multi_waits(nc):
    # Same walrus limitation: at most ONE sync wait per instruction. Move
    # extra waits onto NOPs inserted just before, on the same engine.
    for fn in nc.m.functions:
        for bb in fn.blocks:
            insts = bb.instructions
            i = 0
            while i < len(insts):
                inst = insts[i]
                si = inst.sync_info
                if si is not None and si.on_wait and len(si.on_wait) > 1:
                    extra = list(si.on_wait[1:])
                    si.on_wait = [si.on_wait[0]]
                    for w in extra:
                        nop = mybir.InstNoOp(
                            name=nc.get_next_instruction_name(), ins=[], outs=[]
                        )
                        nop.engine = inst.engine
                        nop.sync_info = mybir.SyncInfo(on_wait=[w], on_update=[])
                        insts.insert(i, nop)
                        i += 1
                i += 1


def build_program():
    nc = bass.Bass("TRN2", debug=False)
    q_t = nc.dram_tensor("q", [SC, D_IN], F32, kind="ExternalInput").ap()
    k_t = nc.dram_tensor("k", [SC, D_IN], F32, kind="ExternalInput").ap()
    v_t = nc.dram_tensor("v", [SC, EV], F32, kind="ExternalInput").ap()
    mem_t = nc.dram_tensor("memory", [EK, EV], F32, kind="ExternalInput").ap()
    zn_t = nc.dram_tensor("z_norm", [1, EK], F32, kind="ExternalInput").ap()
    amem_t = nc.dram_tensor("a_mem", [SC, EV], F32, kind="ExternalOutput").ap()
    nz_t = nc.dram_tensor("nz", [SC, EK], F32, kind="ExternalOutput").ap()
    toadd_t = nc.dram_tensor("toadd", [EK, EV], F32, kind="ExternalOutput").ap()

    with tile.TileContext(nc) as tc:
        with (
            tc.tile_pool(name="consts", bufs=1) as consts,
            tc.tile_pool(name="xin", bufs=2) as xin_pool,
            tc.tile_pool(name="vin", bufs=2) as vin_pool,
            tc.tile_pool(name="xc", bufs=2) as xc_pool,
            tc.tile_pool(name="f", bufs=2) as f_pool,
            tc.tile_pool(name="ft", bufs=2) as ft_pool,
            tc.tile_pool(name="kact", bufs=2) as kact_pool,
            tc.tile_pool(name="w", bufs=2) as w_pool,
            tc.tile_pool(name="outs", bufs=2) as out_pool,
            tc.tile_pool(name="small", bufs=2) as small_pool,
            tc.tile_pool(name="ps_rnum", bufs=2, space="PSUM") as ps_rnum,
            tc.tile_pool(name="ps_ft", bufs=2, space="PSUM") as ps_ft,
            tc.tile_pool(name="ps_acc", bufs=1, space="PSUM") as ps_acc,
        ):
            # ---- constants ----
            # mem_aug[p, h, :] = [memory[h*128+p, :] | z_norm[h*128+p] | 1.0]
            mem_aug = consts.tile([128, 2, EV + 2], F32)
            nc.sync.dma_start(
                out=mem_aug[:, :, 0:EV],
                in_=mem_t.rearrange("(h p) e -> p h e", p=128),
            )
            nc.sync.dma_start(
                out=mem_aug[:, :, EV : EV + 1],
                in_=zn_t.rearrange("o (h p) -> p h o", p=128),
            )
            nc.vector.memset(mem_aug[:, :, EV + 1 : EV + 2], 1.0)

            # z_norm broadcast across partitions: zn_b[p, e] = z_norm[0, e]
            zn_b = consts.tile([128, EK], F32)
            zn_bcast_src = bass.AP(
                tensor=zn_t.tensor,
                offset=zn_t.offset,
                ap=[[0, 128], zn_t.ap[1]],
            )
            nc.gpsimd.dma_start(out=zn_b, in_=zn_bcast_src)

            ident = consts.tile([128, 128], F32)
            make_identity(nc, ident)

            # persistent accumulator: toadd halves in separate PSUM banks
            toadd_ps = ps_acc.tile([128, 2, 512], F32)

            def do_side(x_t, is_k, n_sb_chunks=NCHUNK):
                for sb in range(NSB):
                    X = xin_pool.tile([P, B, D_IN], F32)
                    nc.sync.dma_start(
                        out=X,
                        in_=x_t.rearrange("(s c p) d -> s p c d", c=B, p=P)[sb],
                    )
                    if is_k:
                        V = vin_pool.tile([P, B, EV], F32)
                        nc.sync.dma_start(
                            out=V,
                            in_=v_t.rearrange("(s c p) e -> s p c e", c=B, p=P)[sb],
                        )

                    # xc = [relu(x) | relu(-x)] per chunk
                    XC = xc_pool.tile([P, B, EK], F32)
                    nc.scalar.activation(
                        out=XC[:, :, 0:D_IN], in_=X, func=AFT.Relu, scale=1.0
                    )
                    nc.scalar.activation(
                        out=XC[:, :, D_IN:EK], in_=X, func=AFT.Relu, scale=-1.0
                    )

                    # f = xc * roll(xc, 1) -- shifted slice + wrap column
                    F = f_pool.tile([P, B, EK], F32)
                    nc.vector.tensor_mul(
                        F[:, :, 1:EK], XC[:, :, 1:EK], XC[:, :, 0 : EK - 1]
                    )
                    nc.vector.tensor_mul(
                        F[:, :, 0:1], XC[:, :, 0:1], XC[:, :, EK - 1 : EK]
                    )

                    # transpose f chunks (PE) -> FT[ek_in_half, chunk, half, tok]
                    FT = ft_pool.tile([128, B, 2, P], F32)
                    for g in range(B // 2):
                        pft = ps_ft.tile([128, 512], F32)
                        for j in range(2):
                            c = g * 2 + j
                            for h in range(2):
                                nc.tensor.transpose(
                                    pft[:, (j * 2 + h) * 128 : (j * 2 + h + 1) * 128],
                                    F[:, c, h * 128 : (h + 1) * 128],
                                    ident,
                                )
                        cp_eng = nc.scalar if g % 2 == 0 else nc.vector
                        if cp_eng is nc.scalar:
                            nc.scalar.copy(
                                out=FT[:, g * 2 : g * 2 + 2, :, :], in_=pft
                            )
                        else:
                            nc.vector.tensor_copy(
                                out=FT[:, g * 2 : g * 2 + 2, :, :], in_=pft
                            )

                    DINV = small_pool.tile([P, B], F32, tag="dinv")
                    if is_k:
                        RINV = small_pool.tile([P, B], F32, tag="rinv")
                        W = w_pool.tile([P, B, EV], F32)
                    else:
                        AM = out_pool.tile([P, B, EV], F32, tag="side_out")

                    for pr in range(B // 2):
                        rn = ps_rnum.tile([128, 2, 512], F32)
                        for j in range(2):
                            c = pr * 2 + j
                            nc.tensor.matmul(
                                rn[:, j, 0 : EV + 2],
                                FT[:, c, 0, :],
                                mem_aug[:, 0, :],
                                start=True,
                                stop=False,
                            )
                            nc.tensor.matmul(
                                rn[:, j, 0 : EV + 2],
                                FT[:, c, 1, :],
                                mem_aug[:, 1, :],
                                start=False,
                                stop=True,
                            )
                        sl = slice(pr * 2, pr * 2 + 2)
                        nc.vector.reciprocal(out=DINV[:, sl], in_=rn[:, :, EV])
                        if is_k:
                            nc.vector.reciprocal(
                                out=RINV[:, sl], in_=rn[:, :, EV + 1]
                            )
                            for j in range(2):
                                c = pr * 2 + j
                                # w' = num * dinv - v = r_k - v
                                nc.vector.scalar_tensor_tensor(
                                    out=W[:, c, :],
                                    in0=rn[:, j, 0:EV],
                                    scalar=DINV[:, c : c + 1],
                                    in1=V[:, c, :],
                                    op0=AOP.mult,
                                    op1=AOP.subtract,
                                )
                        else:
                            nc.vector.tensor_mul(
                                AM[:, sl, :],
                                rn[:, :, 0:EV],
                                DINV[:, sl, None].to_broadcast([P, 2, EV]),
                            )

                    if is_k:
                        KACT = kact_pool.tile([P, B, EK], F32)
                        for c in range(B):
                            nc.gpsimd.tensor_scalar_mul(
                                out=KACT[:, c, :],
                                in0=F[:, c, :],
                                scalar1=RINV[:, c : c + 1],
                            )
                        NZ = out_pool.tile([P, B, EK], F32, tag="side_out")
                        nc.gpsimd.tensor_add(
                            out=NZ,
                            in0=KACT,
                            in1=zn_b[:, None, :].to_broadcast([P, B, EK]),
                        )
                        for c in range(B):
                            first = sb == 0 and c == 0
                            last = sb == NSB - 1 and c == B - 1
                            for h in range(2):
                                nc.tensor.matmul(
                                    toadd_ps[:, h, 0:EV],
                                    KACT[:, c, h * 128 : (h + 1) * 128],
                                    W[:, c, :],
                                    start=first,
                                    stop=last,
                                )
                        nc.sync.dma_start(
                            out=nz_t.rearrange("(s c p) e -> s p c e", c=B, p=P)[sb],
                            in_=NZ,
                        )
                    else:
                        nc.sync.dma_start(
                            out=amem_t.rearrange("(s c p) e -> s p c e", c=B, p=P)[
                                sb
                            ],
                            in_=AM,
                        )

            do_side(q_t, is_k=False)
            do_side(k_t, is_k=True)

            TO = out_pool.tile([128, 2, EV], F32, tag="toadd_out")
            nc.vector.tensor_copy(out=TO, in_=toadd_ps[:, :, 0:EV])
            nc.sync.dma_start(
                out=toadd_t.rearrange("(h p) e -> p h e", p=128), in_=TO
            )

    _split_multi_waits(nc)
    return nc


_NC = None


def _get_nc():
    global _NC
    if _NC is None:
        _NC = build_program()
    return _NC


def kernel(q, k, v, memory, z_norm):
    q = np.ascontiguousarray(q, dtype=np.float32)
    k = np.ascontiguousarray(k, dtype=np.float32)
    v = np.ascontiguousarray(v, dtype=np.float32)
    memory = np.ascontiguousarray(memory, dtype=np.float32)
    z_norm = np.ascontiguousarray(z_norm, dtype=np.float32)

    nc = _get_nc()
    in_maps = []
    for c in range(N_CORES):
        sl = slice(c * SC, (c + 1) * SC)
        in_maps.append(
            {
                "q": q[sl],
                "k": k[sl],
                "v": v[sl],
                "memory": memory,
                "z_norm": z_norm,
            }
        )
    res = run_bass_kernel_spmd(nc, in_maps, core_ids=list(range(N_CORES)))
    a_mem = np.concatenate([res.results[c]["a_mem"] for c in range(N_CORES)], axis=0)
    nz = np.concatenate([res.results[c]["nz"] for c in range(N_CORES)], axis=0)
    parts = np.sum([res.results[c]["toadd"] for c in range(N_CORES)], axis=0)
    new_memory = memory - parts  # device computed k_act.T @ (r_k - v)
    return a_mem, new_memory, nz
